# revision 24
# baseline (speedup 1.0000x reference)
"""Multi-head attention (B=8, S=2048, E=1024, H=8, D=128) on 8 Trainium2 cores.

Strategy: data-parallel over batch (one batch element per core, no collectives).

All matmuls run in bf16 with fp32 PSUM accumulation, 256-wide moving tiles
(bf16 at 512-wide measured ~1.6x worse per column on this hardware; 256-wide
measures ~0.85 cyc/col):
  - x^T is loaded once per rep into SBUF as bf16 [128, 8ec, 2048] and stays
    resident; Q^T/K^T per head ([d, s]) use x as the moving operand, V
    ([t, d] pair-packed) uses Wv as the moving operand.
  - Attention runs in the transposed-score layout S^T = [t, s]: softmax exp
    on ACT (PSUM -> SBUF bf16, scale fused), PV contracts t on partitions
    with es as the moving operand.
  - Softmax denominators come from a DVE bf16 add-tree over the es tiles
    (16x fewer PE cycles than a ones-matmul row-sum; one final 128x128 ones
    matmul per (head, s-block) replicates the partition sum), then
    reciprocal + tensor_mul normalize the PV output into AT.
  - Engine FIFOs are in-order, so emission order is schedule order:
    PV groups are software-pipelined one stage behind the scores matmuls
    (never blocking the PE FIFO on ACT), and the next pair's QKV projection
    groups are emitted interleaved between attention units so they fill the
    PE idle inside the ACT-gated attention phase.
  - bq is fused into the Q eviction (per-partition tensor_scalar_add);
    bk is dropped (per-query constant, softmax-invariant); bv is folded into
    bo on the host (sum_t p_t = 1 exactly), and bo arrives pre-broadcast
    [128, E] so the output eviction is a single DVE tensor_add.
PSUM budget (8 banks): st 2x[128,1024] (4) + pv [128,512] (1) + rsb (1) +
qkv acc 2x[128,512] (2).
"""

import numpy as np
from contextlib import ExitStack

import concourse.bass as bass
import concourse.tile as tile
from concourse import bacc, mybir
from concourse.bass_utils import run_bass_kernel_spmd

B = 8
S = 2048
E = 1024
H = 8
D = 128
P = 128
EC = E // P          # 8 contraction chunks over embed
TC = S // P          # 16 t-chunks
NSB = S // 512       # 4 s-blocks of 512
SCALE = 1.0 / float(np.sqrt(D))

F32 = mybir.dt.float32
BF16 = mybir.dt.bfloat16
EXP = mybir.ActivationFunctionType.Exp


def build_nc(reps=1, mmw=256, do_att=1, do_out=1, do_adds=1, do_norm=1,
             qkv_per_unit=3, defer_norm=1, early_w=1, esb=5):
    MW = mmw
    NM = 512 // MW       # matmuls per 512-wide block
    nc = bacc.Bacc(None)

    xTb = nc.dram_tensor("xTb", (E, S), BF16, kind="ExternalInput")
    wqT = nc.dram_tensor("wqT", (E, E), BF16, kind="ExternalInput")
    wkT = nc.dram_tensor("wkT", (E, E), BF16, kind="ExternalInput")
    wvT = nc.dram_tensor("wvT", (E, E), BF16, kind="ExternalInput")
    woT = nc.dram_tensor("woT", (E, E), BF16, kind="ExternalInput")
    bq2 = nc.dram_tensor("bq2", (P, H), F32, kind="ExternalInput")   # bq as [d, h]
    bob = nc.dram_tensor("bob", (P, E), F32, kind="ExternalInput")   # bo+Wo@bv, bcast
    out = nc.dram_tensor("out", (S, E), F32, kind="ExternalOutput")

    x_r = xTb.rearrange("(ec p) s -> p ec s", p=P)
    wq_r = wqT.rearrange("(ec p) d -> p ec d", p=P)
    wk_r = wkT.rearrange("(ec p) d -> p ec d", p=P)
    wv_r = wvT.rearrange("(ec p) d -> p ec d", p=P)
    wo_r = woT.rearrange("(hc p) e -> p hc e", p=P)

    with tile.TileContext(nc) as tc:
        with ExitStack() as octx:
            const = octx.enter_context(tc.tile_pool(name="const", bufs=1))
            atp = octx.enter_context(tc.tile_pool(name="atp", bufs=1))

            ones_f = const.tile([P, P], BF16)
            nc.vector.memset(ones_f, 1.0)
            bq_s = const.tile([P, H], F32)
            nc.sync.dma_start(out=bq_s, in_=bq2[:, :])
            bob_s = const.tile([P, E], F32)
            nc.sync.dma_start(out=bob_s, in_=bob[:, :])

            AT = atp.tile([P, H, S], BF16)  # normalized A^T per head

            for _rep in range(reps):
                with ExitStack() as ctx:
                    xp = ctx.enter_context(tc.tile_pool(name="xp", bufs=1))
                    wpool = ctx.enter_context(tc.tile_pool(name="wpool", bufs=2))
                    qkp = ctx.enter_context(tc.tile_pool(name="qkp", bufs=2))
                    vp = ctx.enter_context(tc.tile_pool(name="vp", bufs=2))
                    esp = ctx.enter_context(tc.tile_pool(name="esp", bufs=esb))
                    addp = ctx.enter_context(tc.tile_pool(name="addp", bufs=2))
                    rcpp = ctx.enter_context(tc.tile_pool(name="rcpp", bufs=2))
                    stp = ctx.enter_context(tc.tile_pool(name="stp", bufs=2, space="PSUM"))
                    attacc = ctx.enter_context(tc.tile_pool(name="attacc", bufs=1, space="PSUM"))
                    qacc = ctx.enter_context(tc.tile_pool(name="qacc", bufs=2, space="PSUM"))

                    x_sb = xp.tile([P, EC, S], BF16, tag="x")

                    def emit_x():
                        for sb in range(NSB):
                            nc.sync.dma_start(
                                out=x_sb[:, :, sb * 512:(sb + 1) * 512],
                                in_=x_r[:, :, sb * 512:(sb + 1) * 512])
                    if not early_w:
                        emit_x()

                    def setup_pair(pair):
                        """Alloc tiles, DMA weights, and return the pair's QKV
                        projection work as a list of closures (one PSUM-group
                        each) to be drained between attention units."""
                        h0 = 2 * pair
                        wq01 = wpool.tile([P, EC, 2 * D], BF16, tag="wq",
                                          name=f"wq01_{pair}")
                        wk01 = wpool.tile([P, EC, 2 * D], BF16, tag="wk",
                                          name=f"wk01_{pair}")
                        wv01 = wpool.tile([P, EC, 2 * D], BF16, tag="wv",
                                          name=f"wv01_{pair}")
                        nc.sync.dma_start(out=wq01, in_=wq_r[:, :, h0 * D:(h0 + 2) * D])
                        nc.sync.dma_start(out=wk01, in_=wk_r[:, :, h0 * D:(h0 + 2) * D])
                        nc.sync.dma_start(out=wv01, in_=wv_r[:, :, h0 * D:(h0 + 2) * D])

                        qt0 = qkp.tile([P, S], BF16, tag="qt0", name=f"qt0_{pair}")
                        qt1 = qkp.tile([P, S], BF16, tag="qt1", name=f"qt1_{pair}")
                        kt0 = qkp.tile([P, S], BF16, tag="kt0", name=f"kt0_{pair}")
                        kt1 = qkp.tile([P, S], BF16, tag="kt1", name=f"kt1_{pair}")
                        vv01 = vp.tile([P, TC, 2 * D], BF16, tag="vv",
                                       name=f"vv01_{pair}")

                        work = []

                        def qk_group(sb, w, dest, hh, isq):
                            def emit():
                                s0 = sb * 512
                                ps = qacc.tile([P, 512], F32, tag="acc",
                                               name=f"ps_qk{pair}_{sb}_{hh}_{isq}")
                                for m in range(NM):
                                    for ec in range(EC):
                                        nc.tensor.matmul(
                                            ps[:, m * MW:(m + 1) * MW],
                                            w[:, ec, hh * D:(hh + 1) * D],
                                            x_sb[:, ec, s0 + m * MW:s0 + (m + 1) * MW],
                                            start=(m == 0 and ec == 0),
                                            stop=(m == NM - 1 and ec == EC - 1),
                                        )
                                if isq:
                                    nc.vector.tensor_scalar_add(
                                        dest[:, s0:s0 + 512], ps,
                                        bq_s[:, h0 + hh:h0 + hh + 1])
                                else:
                                    nc.vector.tensor_copy(dest[:, s0:s0 + 512], ps)
                            return emit

                        def v_group(sb, tcp):
                            def emit():
                                ps = qacc.tile([P, 512], F32, tag="acc",
                                               name=f"ps_v{pair}_{sb}_{tcp}")
                                for j in range(2):
                                    t0 = (sb * 4 + tcp * 2 + j) * P
                                    for ec in range(EC):
                                        nc.tensor.matmul(
                                            ps[:, j * 256:(j + 1) * 256],
                                            x_sb[:, ec, t0:t0 + P],
                                            wv01[:, ec, :],
                                            start=(j == 0 and ec == 0),
                                            stop=(j == 1 and ec == EC - 1),
                                        )
                                tci = sb * 4 + tcp * 2
                                nc.vector.tensor_copy(vv01[:, tci:tci + 2, :], ps)
                            return emit

                        for sb in range(NSB):
                            for w, dest, hh, isq in (
                                    (wq01, qt0, 0, True), (wq01, qt1, 1, True),
                                    (wk01, kt0, 0, False), (wk01, kt1, 1, False)):
                                work.append(qk_group(sb, w, dest, hh, isq))
                            for tcp in range(2):
                                work.append(v_group(sb, tcp))
                        return (qt0, qt1, kt0, kt1, vv01), work

                    pend_norm = []   # deferred normalization closures

                    def attention_pair(pair, tiles, fill_work):
                        """Emit attention for both heads of `pair`, draining
                        `fill_work` closures between units to keep PE fed.
                        With defer_norm, each unit's add-tree tail (rsb
                        matmul, reciprocal, multiply) is emitted a few
                        stages into the NEXT unit so the in-order PE FIFO
                        never parks on the DVE chain."""
                        qt0, qt1, kt0, kt1, vv01 = tiles
                        h0 = 2 * pair
                        for hi in range(2):
                            h = h0 + hi
                            qt = (qt0, qt1)[hi]
                            kt = (kt0, kt1)[hi]
                            for sb in range(NSB):
                                s0 = sb * 512
                                ps_pv = attacc.tile([P, 512], F32, tag="pv",
                                                    bufs=(2 if defer_norm else 1),
                                                    name=f"pv{pair}_{hi}_{sb}")
                                es_l = []
                                a_l = []

                                def emit_pv(tcp):
                                    es = es_l[tcp]
                                    for j in range(2):
                                        tc_i = tcp * 2 + j
                                        for m in range(NM):
                                            nc.tensor.matmul(
                                                ps_pv[:, m * MW:(m + 1) * MW],
                                                vv01[:, tc_i, hi * D:(hi + 1) * D],
                                                es[:, j * 512 + m * MW:j * 512 + (m + 1) * MW],
                                                start=(tc_i == 0 and m == 0),
                                                stop=(tc_i == TC - 1 and m == NM - 1),
                                            )

                                for tcp in range(TC // 2):
                                    st = stp.tile([P, 1024], F32, tag="st",
                                                  name=f"st{pair}_{hi}_{sb}_{tcp}")
                                    for j in range(2):
                                        tc_i = tcp * 2 + j
                                        for m in range(NM):
                                            nc.tensor.matmul(
                                                st[:, j * 512 + m * MW:j * 512 + (m + 1) * MW],
                                                kt[:, tc_i * P:(tc_i + 1) * P],
                                                qt[:, s0 + m * MW:s0 + (m + 1) * MW],
                                                start=(m == 0), stop=(m == NM - 1),
                                            )
                                    es = esp.tile([P, 1024], BF16, tag="es", bufs=esb,
                                                  name=f"es{pair}_{hi}_{sb}_{tcp}")
                                    nc.scalar.activation(es, st, EXP, scale=SCALE)
                                    es_l.append(es)
                                    # PV one stage behind scores: the PE FIFO
                                    # never parks on an un-finished exp.
                                    if tcp > 0:
                                        emit_pv(tcp - 1)
                                    if tcp == 3 and pend_norm:
                                        pend_norm.pop(0)()
                                    if do_adds and tcp % 2 == 1:
                                        a = addp.tile([P, 1024], BF16, tag="a", bufs=8,
                                                      name=f"a{pair}_{hi}_{sb}_{tcp}")
                                        nc.vector.tensor_add(a, es_l[tcp - 1], es_l[tcp])
                                        a_l.append(a)
                                emit_pv(TC // 2 - 1)

                                if not do_adds or not do_norm:
                                    nc.vector.tensor_copy(AT[:, h, s0:s0 + 512], ps_pv)
                                else:
                                    def norm(pair=pair, hi=hi, h=h, sb=sb, s0=s0,
                                             a_l=a_l, ps_pv=ps_pv):
                                        b0 = addp.tile([P, 1024], BF16, tag="b",
                                                       name=f"b0{pair}_{hi}_{sb}")
                                        b1 = addp.tile([P, 1024], BF16, tag="b1",
                                                       name=f"b1{pair}_{hi}_{sb}")
                                        nc.vector.tensor_add(b0, a_l[0], a_l[1])
                                        nc.vector.tensor_add(b1, a_l[2], a_l[3])
                                        cc = addp.tile([P, 1024], BF16, tag="c",
                                                       name=f"c{pair}_{hi}_{sb}")
                                        nc.vector.tensor_add(cc, b0, b1)
                                        racc = addp.tile([P, 512], BF16, tag="racc",
                                                         name=f"racc{pair}_{hi}_{sb}")
                                        nc.vector.tensor_add(racc, cc[:, 0:512],
                                                             cc[:, 512:1024])
                                        if defer_norm:
                                            rsb = qacc.tile([P, 512], F32, tag="acc",
                                                            name=f"rsb{pair}_{hi}_{sb}")
                                        else:
                                            rsb = attacc.tile([P, 512], F32, tag="rsb",
                                                              name=f"rsb{pair}_{hi}_{sb}")
                                        nc.tensor.matmul(rsb, ones_f, racc,
                                                         start=True, stop=True)
                                        rcp = rcpp.tile([P, 512], F32, tag="rcp",
                                                        name=f"rcp{pair}_{hi}_{sb}")
                                        nc.vector.reciprocal(rcp, rsb)
                                        nc.vector.tensor_mul(AT[:, h, s0:s0 + 512],
                                                             ps_pv, rcp)
                                    if defer_norm:
                                        pend_norm.append(norm)
                                    else:
                                        norm()
                                # fill PE with next pair's QKV between units
                                for _ in range(qkv_per_unit):
                                    if fill_work:
                                        fill_work.pop(0)()

                    tiles, work = setup_pair(0)
                    if early_w:
                        emit_x()   # weight DMAs queue ahead of the 4MB x load
                    for w in work:
                        w()
                    for pair in range(H // 2):
                        if pair + 1 < H // 2:
                            next_tiles, next_work = setup_pair(pair + 1)
                        else:
                            next_tiles, next_work = None, []
                        if do_att:
                            attention_pair(pair, tiles, next_work)
                        for w in next_work:   # drain leftovers
                            w()
                        tiles = next_tiles
                    while pend_norm:
                        pend_norm.pop(0)()

                # ---- output projection ----
                if not do_out:
                    continue
                with ExitStack() as ctx:
                    wop = ctx.enter_context(tc.tile_pool(name="wop", bufs=1))
                    outp = ctx.enter_context(tc.tile_pool(name="outp", bufs=3))
                    pso = ctx.enter_context(tc.tile_pool(name="pso", bufs=2, space="PSUM"))

                    wo = wop.tile([P, H, E], BF16)
                    nc.sync.dma_start(out=wo, in_=wo_r[:, :, :])

                    for sc in range(S // P):
                        ps = pso.tile([P, E], F32, tag="po", name=f"po{sc}")
                        for hc in range(H):
                            for eb in range(E // MW):
                                nc.tensor.matmul(
                                    ps[:, eb * MW:(eb + 1) * MW],
                                    AT[:, hc, sc * P:(sc + 1) * P],
                                    wo[:, hc, eb * MW:(eb + 1) * MW],
                                    start=(hc == 0 and eb % NM == 0),
                                    stop=(hc == H - 1 and eb % NM == NM - 1),
                                )
                        ot = outp.tile([P, E], F32, tag="ot", name=f"ot{sc}")
                        nc.vector.tensor_add(ot, ps, bob_s)
                        nc.sync.dma_start(out=out[sc * P:(sc + 1) * P, :], in_=ot)

    nc.compile()
    return nc


_NC_CACHE = []


def _get_nc():
    if not _NC_CACHE:
        _NC_CACHE.append(build_nc())
    return _NC_CACHE[0]


def _bf16(a):
    import ml_dtypes
    return np.ascontiguousarray(np.asarray(a, np.float32).astype(ml_dtypes.bfloat16))


def prep_inmaps(hidden_state, Wq, bq, Wk, bk, Wv, bv, Wo, bo):
    hidden_state = np.ascontiguousarray(hidden_state, dtype=np.float32)
    Wo32 = np.asarray(Wo, np.float32)
    wqT = _bf16(np.asarray(Wq, np.float32).reshape(E, E).T)
    wkT = _bf16(np.asarray(Wk, np.float32).reshape(E, E).T)
    wvT = _bf16(np.asarray(Wv, np.float32).reshape(E, E).T)
    woT = _bf16(Wo32.T)
    bq2a = np.ascontiguousarray(np.asarray(bq, np.float32).reshape(H, D).T)
    # bk is a per-query constant shift through softmax (exact no-op).
    # bv folds into bo because sum_t softmax = 1: bo' = bo + Wo @ concat(bv).
    bo_eff = (np.asarray(bo, np.float32)
              + Wo32 @ np.asarray(bv, np.float32).reshape(E)).astype(np.float32)
    bob = np.ascontiguousarray(np.broadcast_to(bo_eff[None, :], (P, E)))
    in_maps = []
    for c in range(B):
        xTbc = _bf16(hidden_state[c].T)
        in_maps.append({
            "xTb": xTbc, "wqT": wqT, "wkT": wkT, "wvT": wvT, "woT": woT,
            "bq2": bq2a, "bob": bob,
        })
    return in_maps


def kernel(hidden_state, Wq, bq, Wk, bk, Wv, bv, Wo, bo):
    in_maps = prep_inmaps(hidden_state, Wq, bq, Wk, bk, Wv, bv, Wo, bo)
    nc = _get_nc()
    res = run_bass_kernel_spmd(nc, in_maps, core_ids=list(range(B)))
    return np.stack([res.results[c]["out"] for c in range(B)])


# revision 26
# speedup vs baseline: 1.2824x; 1.2824x over previous
"""Multi-head attention (B=8, S=2048, E=1024, H=8, D=128) on 8 Trainium2 cores.

Strategy: data-parallel over batch (one batch element per core, no collectives).

All matmuls run in bf16 with fp32 PSUM accumulation, 256-wide moving tiles
(bf16 at 512-wide measured ~1.6x worse per column on this hardware; 256-wide
measures ~0.85 cyc/col):
  - x^T is loaded once per rep into SBUF as bf16 [128, 8ec, 2048] and stays
    resident; Q^T/K^T per head ([d, s]) use x as the moving operand, V
    ([t, d] pair-packed) uses Wv as the moving operand.
  - Attention runs in the transposed-score layout S^T = [t, s]: softmax exp
    on ACT (PSUM -> SBUF bf16, scale fused), PV contracts t on partitions
    with es as the moving operand.
  - Softmax denominators come from a DVE bf16 add-tree over the es tiles
    (16x fewer PE cycles than a ones-matmul row-sum; one final 128x128 ones
    matmul per (head, s-block) replicates the partition sum), then
    reciprocal + tensor_mul normalize the PV output into AT.
  - Engine FIFOs are in-order, so emission order is schedule order:
    PV groups are software-pipelined one stage behind the scores matmuls
    (never blocking the PE FIFO on ACT), and the next pair's QKV projection
    groups are emitted interleaved between attention units so they fill the
    PE idle inside the ACT-gated attention phase.
  - bq is fused into the Q eviction (per-partition tensor_scalar_add);
    bk is dropped (per-query constant, softmax-invariant); bv is folded into
    bo on the host (sum_t p_t = 1 exactly), and bo arrives pre-broadcast
    [128, E] so the output eviction is a single DVE tensor_add.
  - defer_norm: each unit's normalization tail (add-tree -> rsb ones-matmul
    -> reciprocal -> multiply) is emitted a few stages into the NEXT unit,
    so the in-order PE FIFO never parks waiting for the DVE chain to drain
    (worth ~100-300us). The "a" add ring holds two units' worth of buffers
    (bufs=8) to avoid a ring-reuse deadlock across deferred consumers.
PSUM budget (8 banks): st 2x[128,1024] (4) + pv 2x[128,512] (2) +
qkv/rsb acc ring 2x[128,512] (2).
"""

import numpy as np
from contextlib import ExitStack

import concourse.bass as bass
import concourse.tile as tile
from concourse import bacc, mybir
from concourse.bass_utils import run_bass_kernel_spmd

B = 8
S = 2048
E = 1024
H = 8
D = 128
P = 128
EC = E // P          # 8 contraction chunks over embed
TC = S // P          # 16 t-chunks
NSB = S // 512       # 4 s-blocks of 512
SCALE = 1.0 / float(np.sqrt(D))

F32 = mybir.dt.float32
BF16 = mybir.dt.bfloat16
EXP = mybir.ActivationFunctionType.Exp


def build_nc(reps=1, mmw=256, do_att=1, do_out=1, do_adds=1, do_norm=1,
             qkv_per_unit=3, defer_norm=1, early_w=1, esb=5):
    MW = mmw
    NM = 512 // MW       # matmuls per 512-wide block
    nc = bacc.Bacc(None)

    xTb = nc.dram_tensor("xTb", (E, S), BF16, kind="ExternalInput")
    wqT = nc.dram_tensor("wqT", (E, E), BF16, kind="ExternalInput")
    wkT = nc.dram_tensor("wkT", (E, E), BF16, kind="ExternalInput")
    wvT = nc.dram_tensor("wvT", (E, E), BF16, kind="ExternalInput")
    woT = nc.dram_tensor("woT", (E, E), BF16, kind="ExternalInput")
    bq2 = nc.dram_tensor("bq2", (P, H), F32, kind="ExternalInput")   # bq as [d, h]
    bob = nc.dram_tensor("bob", (P, E), F32, kind="ExternalInput")   # bo+Wo@bv, bcast
    out = nc.dram_tensor("out", (S, E), F32, kind="ExternalOutput")

    x_r = xTb.rearrange("(ec p) s -> p ec s", p=P)
    wq_r = wqT.rearrange("(ec p) d -> p ec d", p=P)
    wk_r = wkT.rearrange("(ec p) d -> p ec d", p=P)
    wv_r = wvT.rearrange("(ec p) d -> p ec d", p=P)
    wo_r = woT.rearrange("(hc p) e -> p hc e", p=P)

    with tile.TileContext(nc) as tc:
        with ExitStack() as octx:
            const = octx.enter_context(tc.tile_pool(name="const", bufs=1))
            atp = octx.enter_context(tc.tile_pool(name="atp", bufs=1))

            ones_f = const.tile([P, P], BF16)
            nc.vector.memset(ones_f, 1.0)
            bq_s = const.tile([P, H], F32)
            nc.sync.dma_start(out=bq_s, in_=bq2[:, :])
            bob_s = const.tile([P, E], F32)
            nc.sync.dma_start(out=bob_s, in_=bob[:, :])

            AT = atp.tile([P, H, S], BF16)  # normalized A^T per head

            for _rep in range(reps):
                with ExitStack() as ctx:
                    xp = ctx.enter_context(tc.tile_pool(name="xp", bufs=1))
                    wpool = ctx.enter_context(tc.tile_pool(name="wpool", bufs=2))
                    qkp = ctx.enter_context(tc.tile_pool(name="qkp", bufs=2))
                    vp = ctx.enter_context(tc.tile_pool(name="vp", bufs=2))
                    esp = ctx.enter_context(tc.tile_pool(name="esp", bufs=esb))
                    addp = ctx.enter_context(tc.tile_pool(name="addp", bufs=2))
                    rcpp = ctx.enter_context(tc.tile_pool(name="rcpp", bufs=2))
                    wop = ctx.enter_context(tc.tile_pool(name="wop", bufs=1))
                    outp2 = ctx.enter_context(tc.tile_pool(name="outp2", bufs=3))
                    stp = ctx.enter_context(tc.tile_pool(name="stp", bufs=2, space="PSUM"))
                    attacc = ctx.enter_context(tc.tile_pool(name="attacc", bufs=1, space="PSUM"))
                    qacc = ctx.enter_context(tc.tile_pool(name="qacc", bufs=2, space="PSUM"))

                    x_sb = xp.tile([P, EC, S], BF16, tag="x")

                    def emit_x():
                        for sb in range(NSB):
                            nc.sync.dma_start(
                                out=x_sb[:, :, sb * 512:(sb + 1) * 512],
                                in_=x_r[:, :, sb * 512:(sb + 1) * 512])
                    if not early_w:
                        emit_x()

                    def setup_pair(pair):
                        """Alloc tiles, DMA weights, and return the pair's QKV
                        projection work as a list of closures (one PSUM-group
                        each) to be drained between attention units."""
                        h0 = 2 * pair
                        wq01 = wpool.tile([P, EC, 2 * D], BF16, tag="wq",
                                          name=f"wq01_{pair}")
                        wk01 = wpool.tile([P, EC, 2 * D], BF16, tag="wk",
                                          name=f"wk01_{pair}")
                        wv01 = wpool.tile([P, EC, 2 * D], BF16, tag="wv",
                                          name=f"wv01_{pair}")
                        nc.sync.dma_start(out=wq01, in_=wq_r[:, :, h0 * D:(h0 + 2) * D])
                        nc.sync.dma_start(out=wk01, in_=wk_r[:, :, h0 * D:(h0 + 2) * D])
                        nc.sync.dma_start(out=wv01, in_=wv_r[:, :, h0 * D:(h0 + 2) * D])

                        qt0 = qkp.tile([P, S], BF16, tag="qt0", name=f"qt0_{pair}")
                        qt1 = qkp.tile([P, S], BF16, tag="qt1", name=f"qt1_{pair}")
                        kt0 = qkp.tile([P, S], BF16, tag="kt0", name=f"kt0_{pair}")
                        kt1 = qkp.tile([P, S], BF16, tag="kt1", name=f"kt1_{pair}")
                        vv01 = vp.tile([P, TC, 2 * D], BF16, tag="vv",
                                       name=f"vv01_{pair}")

                        work = []

                        def qk_group(sb, w, dest, hh, isq):
                            def emit():
                                s0 = sb * 512
                                ps = qacc.tile([P, 512], F32, tag="acc",
                                               name=f"ps_qk{pair}_{sb}_{hh}_{isq}")
                                for m in range(NM):
                                    for ec in range(EC):
                                        nc.tensor.matmul(
                                            ps[:, m * MW:(m + 1) * MW],
                                            w[:, ec, hh * D:(hh + 1) * D],
                                            x_sb[:, ec, s0 + m * MW:s0 + (m + 1) * MW],
                                            start=(m == 0 and ec == 0),
                                            stop=(m == NM - 1 and ec == EC - 1),
                                        )
                                if isq:
                                    nc.vector.tensor_scalar_add(
                                        dest[:, s0:s0 + 512], ps,
                                        bq_s[:, h0 + hh:h0 + hh + 1])
                                else:
                                    nc.vector.tensor_copy(dest[:, s0:s0 + 512], ps)
                            return emit

                        def v_group(sb, tcp):
                            def emit():
                                ps = qacc.tile([P, 512], F32, tag="acc",
                                               name=f"ps_v{pair}_{sb}_{tcp}")
                                for j in range(2):
                                    t0 = (sb * 4 + tcp * 2 + j) * P
                                    for ec in range(EC):
                                        nc.tensor.matmul(
                                            ps[:, j * 256:(j + 1) * 256],
                                            x_sb[:, ec, t0:t0 + P],
                                            wv01[:, ec, :],
                                            start=(j == 0 and ec == 0),
                                            stop=(j == 1 and ec == EC - 1),
                                        )
                                tci = sb * 4 + tcp * 2
                                nc.vector.tensor_copy(vv01[:, tci:tci + 2, :], ps)
                            return emit

                        for sb in range(NSB):
                            for w, dest, hh, isq in (
                                    (wq01, qt0, 0, True), (wq01, qt1, 1, True),
                                    (wk01, kt0, 0, False), (wk01, kt1, 1, False)):
                                work.append(qk_group(sb, w, dest, hh, isq))
                            for tcp in range(2):
                                work.append(v_group(sb, tcp))
                        return (qt0, qt1, kt0, kt1, vv01), work

                    pend_norm = []   # deferred normalization closures

                    def attention_pair(pair, tiles, fill_work):
                        """Emit attention for both heads of `pair`, draining
                        `fill_work` closures between units to keep PE fed.
                        With defer_norm, each unit's add-tree tail (rsb
                        matmul, reciprocal, multiply) is emitted a few
                        stages into the NEXT unit so the in-order PE FIFO
                        never parks on the DVE chain."""
                        qt0, qt1, kt0, kt1, vv01 = tiles
                        h0 = 2 * pair
                        for hi in range(2):
                            h = h0 + hi
                            qt = (qt0, qt1)[hi]
                            kt = (kt0, kt1)[hi]
                            for sb in range(NSB):
                                s0 = sb * 512
                                ps_pv = attacc.tile([P, 512], F32, tag="pv",
                                                    bufs=(2 if defer_norm else 1),
                                                    name=f"pv{pair}_{hi}_{sb}")
                                es_l = []
                                a_l = []

                                def emit_pv(tcp):
                                    es = es_l[tcp]
                                    for j in range(2):
                                        tc_i = tcp * 2 + j
                                        for m in range(NM):
                                            nc.tensor.matmul(
                                                ps_pv[:, m * MW:(m + 1) * MW],
                                                vv01[:, tc_i, hi * D:(hi + 1) * D],
                                                es[:, j * 512 + m * MW:j * 512 + (m + 1) * MW],
                                                start=(tc_i == 0 and m == 0),
                                                stop=(tc_i == TC - 1 and m == NM - 1),
                                            )

                                for tcp in range(TC // 2):
                                    st = stp.tile([P, 1024], F32, tag="st",
                                                  name=f"st{pair}_{hi}_{sb}_{tcp}")
                                    for j in range(2):
                                        tc_i = tcp * 2 + j
                                        for m in range(NM):
                                            nc.tensor.matmul(
                                                st[:, j * 512 + m * MW:j * 512 + (m + 1) * MW],
                                                kt[:, tc_i * P:(tc_i + 1) * P],
                                                qt[:, s0 + m * MW:s0 + (m + 1) * MW],
                                                start=(m == 0), stop=(m == NM - 1),
                                            )
                                    es = esp.tile([P, 1024], BF16, tag="es", bufs=esb,
                                                  name=f"es{pair}_{hi}_{sb}_{tcp}")
                                    nc.scalar.activation(es, st, EXP, scale=SCALE)
                                    es_l.append(es)
                                    # PV one stage behind scores: the PE FIFO
                                    # never parks on an un-finished exp.
                                    if tcp > 0:
                                        emit_pv(tcp - 1)
                                    if tcp == 3 and pend_norm:
                                        pend_norm.pop(0)()
                                    if tcp == 5:
                                        for _ in range(8):
                                            if op_work:
                                                op_work.pop(0)()
                                    if do_adds and tcp % 2 == 1:
                                        a = addp.tile([P, 1024], BF16, tag="a", bufs=8,
                                                      name=f"a{pair}_{hi}_{sb}_{tcp}")
                                        nc.vector.tensor_add(a, es_l[tcp - 1], es_l[tcp])
                                        a_l.append(a)
                                emit_pv(TC // 2 - 1)

                                if not do_adds or not do_norm:
                                    nc.vector.tensor_copy(AT[:, h, s0:s0 + 512], ps_pv)
                                else:
                                    def norm(pair=pair, hi=hi, h=h, sb=sb, s0=s0,
                                             a_l=a_l, ps_pv=ps_pv):
                                        b0 = addp.tile([P, 1024], BF16, tag="b",
                                                       name=f"b0{pair}_{hi}_{sb}")
                                        b1 = addp.tile([P, 1024], BF16, tag="b1",
                                                       name=f"b1{pair}_{hi}_{sb}")
                                        nc.vector.tensor_add(b0, a_l[0], a_l[1])
                                        nc.vector.tensor_add(b1, a_l[2], a_l[3])
                                        cc = addp.tile([P, 1024], BF16, tag="c",
                                                       name=f"c{pair}_{hi}_{sb}")
                                        nc.vector.tensor_add(cc, b0, b1)
                                        racc = addp.tile([P, 512], BF16, tag="racc",
                                                         name=f"racc{pair}_{hi}_{sb}")
                                        nc.vector.tensor_add(racc, cc[:, 0:512],
                                                             cc[:, 512:1024])
                                        if defer_norm:
                                            rsb = qacc.tile([P, 512], F32, tag="acc",
                                                            name=f"rsb{pair}_{hi}_{sb}")
                                        else:
                                            rsb = attacc.tile([P, 512], F32, tag="rsb",
                                                              name=f"rsb{pair}_{hi}_{sb}")
                                        nc.tensor.matmul(rsb, ones_f, racc,
                                                         start=True, stop=True)
                                        rcp = rcpp.tile([P, 512], F32, tag="rcp",
                                                        name=f"rcp{pair}_{hi}_{sb}")
                                        nc.vector.reciprocal(rcp, rsb)
                                        nc.vector.tensor_mul(AT[:, h, s0:s0 + 512],
                                                             ps_pv, rcp)
                                        if do_out and h == H - 1:
                                            op_work.extend(outproj_sb(sb))
                                    if defer_norm:
                                        pend_norm.append(norm)
                                    else:
                                        norm()
                                # fill PE with next pair's QKV between units
                                for _ in range(qkv_per_unit):
                                    if fill_work:
                                        fill_work.pop(0)()

                    tiles, work = setup_pair(0)
                    if early_w:
                        emit_x()   # weight DMAs queue ahead of the 4MB x load
                    wo = wop.tile([P, H, E], BF16, tag="wo")
                    if do_out:
                        nc.sync.dma_start(out=wo, in_=wo_r[:, :, :])
                    op_work = []   # outproj chunk closures, legal once h7 lands

                    def outproj_sb(sb):
                        cls = []
                        for sc in range(sb * 4, sb * 4 + 4):
                            for eb in range(2):
                                def emit(sc=sc, eb=eb):
                                    ps = qacc.tile([P, 512], F32, tag="acc",
                                                   name=f"po{sc}_{eb}")
                                    for hc in range(H):
                                        for m in range(NM):
                                            nc.tensor.matmul(
                                                ps[:, m * MW:(m + 1) * MW],
                                                AT[:, hc, sc * P:(sc + 1) * P],
                                                wo[:, hc, eb * 512 + m * MW:eb * 512 + (m + 1) * MW],
                                                start=(hc == 0 and m == 0),
                                                stop=(hc == H - 1 and m == NM - 1),
                                            )
                                    ot = outp2.tile([P, 512], F32, tag="ot",
                                                    name=f"ot{sc}_{eb}")
                                    nc.vector.tensor_add(
                                        ot, ps, bob_s[:, eb * 512:(eb + 1) * 512])
                                    nc.sync.dma_start(
                                        out=out[sc * P:(sc + 1) * P,
                                                eb * 512:(eb + 1) * 512],
                                        in_=ot)
                                cls.append(emit)
                        return cls

                    for w in work:
                        w()
                    for pair in range(H // 2):
                        if pair + 1 < H // 2:
                            next_tiles, next_work = setup_pair(pair + 1)
                        else:
                            next_tiles, next_work = None, []
                        if do_att:
                            attention_pair(pair, tiles, next_work)
                        for w in next_work:   # drain leftovers
                            w()
                        tiles = next_tiles
                    while pend_norm:
                        pend_norm.pop(0)()
                    while op_work:
                        op_work.pop(0)()

    nc.compile()
    return nc


_NC_CACHE = []


def _get_nc():
    if not _NC_CACHE:
        _NC_CACHE.append(build_nc())
    return _NC_CACHE[0]


def _bf16(a):
    import ml_dtypes
    return np.ascontiguousarray(np.asarray(a, np.float32).astype(ml_dtypes.bfloat16))


def prep_inmaps(hidden_state, Wq, bq, Wk, bk, Wv, bv, Wo, bo):
    hidden_state = np.ascontiguousarray(hidden_state, dtype=np.float32)
    Wo32 = np.asarray(Wo, np.float32)
    wqT = _bf16(np.asarray(Wq, np.float32).reshape(E, E).T)
    wkT = _bf16(np.asarray(Wk, np.float32).reshape(E, E).T)
    wvT = _bf16(np.asarray(Wv, np.float32).reshape(E, E).T)
    woT = _bf16(Wo32.T)
    bq2a = np.ascontiguousarray(np.asarray(bq, np.float32).reshape(H, D).T)
    # bk is a per-query constant shift through softmax (exact no-op).
    # bv folds into bo because sum_t softmax = 1: bo' = bo + Wo @ concat(bv).
    bo_eff = (np.asarray(bo, np.float32)
              + Wo32 @ np.asarray(bv, np.float32).reshape(E)).astype(np.float32)
    bob = np.ascontiguousarray(np.broadcast_to(bo_eff[None, :], (P, E)))
    in_maps = []
    for c in range(B):
        xTbc = _bf16(hidden_state[c].T)
        in_maps.append({
            "xTb": xTbc, "wqT": wqT, "wkT": wkT, "wvT": wvT, "woT": woT,
            "bq2": bq2a, "bob": bob,
        })
    return in_maps


def kernel(hidden_state, Wq, bq, Wk, bk, Wv, bv, Wo, bo):
    in_maps = prep_inmaps(hidden_state, Wq, bq, Wk, bk, Wv, bv, Wo, bo)
    nc = _get_nc()
    res = run_bass_kernel_spmd(nc, in_maps, core_ids=list(range(B)))
    return np.stack([res.results[c]["out"] for c in range(B)])
